# revision 33
# baseline (speedup 1.0000x reference)
"""Transformer block (post-LN, BERT-style) on 8 TRN2 NeuronCores, collective-free.

Sharding: 8 cores = 4 batches x 2 query-halves. Core c=(b,j) computes, for
batch b: K/V for all 2048 tokens (recomputed per pair), Q/attention/Wo/FFN
for its own 1024 query tokens. Host concatenates 8 [1024,1024] outputs.

Speed scheme vs the bf16 baseline:
  - Attention-path matmuls (Q/K/V proj, ctx, Wo) run fp8e4 with
    perf_mode=DoubleRow (2 contraction subtiles per pass); a host-side
    x16 weight scale keeps fp8e4 in its normal range, and the x16 cancels
    exactly through the layernorms (LN is scale-invariant).
  - The FFN stays bf16 on both sides: an error simulation shows fp8 in
    the x1->W1->gelu->W2 path alone costs ~2.4e-2 relative error while
    fp8 everywhere in attention costs ~nothing (attention output is small
    against the residual; probs/V errors average out).
  - V proj and the previous chunk's Wo+LN1 are emitted as fillers inside
    the attention slot loop so the PE never idles long enough for the HAM
    clock gate to re-throttle while ACT grinds through 33M softmax exps.
  - LN rsqrt = ACT Sqrt batched per query chunk + one tiny DVE reciprocal,
    so the sqrt<->exp table switch happens twice per chunk, not per tile.
  - Softmax row-sums come from a ones-column in V (free in the ctx matmul);
    reciprocals are batched 4 rows per DVE call at partitions {0,32,64,96}
    so the 1/rowsum broadcast matmuls keep 32-aligned bases.
"""

import math
import os
import sys
import types
import numpy as np

import concourse.bacc as bacc
import concourse.bass as bass
import concourse.tile as tile
import concourse.mybir as mybir
from concourse.bass_utils import run_bass_kernel_spmd

P = 128
F32 = mybir.dt.float32
BF16 = mybir.dt.bfloat16
FP8 = mybir.dt.float8e4
AF = mybir.ActivationFunctionType
ALU = mybir.AluOpType
DR = mybir.MatmulPerfMode.DoubleRow

SCALE = 16.0          # host-side weight/residual scale for fp8 range
QK_SCALE = 0.35355339 / SCALE   # 1/sqrt(sqrt(HD)) folded into q and k


def build_block(nc, *, S, H, NH, FF, eps=1e-12, flags=None, prefix=""):
    flags = flags or set()
    HD = 64
    SQ = S // 2
    HT = H // P            # 8 H-subtiles
    KT = S // P            # 16 k-token tiles
    QC = 512               # query chunk
    NQC = SQ // QC         # 2
    TC = 512
    FT = FF // P           # 32 FF-subtiles
    G = 4                  # kt per probs group
    NG = KT // G           # 4 groups per pair
    NP = NH // 2           # 8 head pairs
    NTT = QC // P          # 4 token tiles per chunk

    def pn(n):
        return f"{prefix}{n}"

    def param(name, shape, dt=F32):
        return nc.declare_dram_parameter(pn(name), list(shape), dt,
                                         isOutput=False)

    xT = param("xT", [H, S], FP8)
    xqT = param("xqT", [H, SQ], FP8)
    xh = param("xh", [SQ, H], BF16)      # 16*(x+bo)
    wq = param("wq", [H, H], FP8)
    wk = param("wk", [H, H], FP8)
    wv = param("wv", [H, H], FP8)
    wo = param("wo", [H, H], FP8)
    w1 = param("w1", [H, FF], BF16)
    w2 = param("w2", [FF, H], BF16)
    opt = {}
    for name, shape in [("mask", [S]), ("bq", [H]), ("bk", [H]), ("bv", [H]),
                        ("b1", [FF]), ("b2", [H]),
                        ("ln1_g", [H]), ("ln1_b", [H]),
                        ("ln2_g", [H]), ("ln2_b", [H])]:
        if name in flags:
            opt[name] = param(name, shape)
    out_ext = nc.declare_dram_parameter(pn("out"), [SQ, H], F32, isOutput=True)

    with (
        tile.TileContext(nc) as tc,
        tc.tile_pool(name=pn("singles"), bufs=1) as singles,
        tc.tile_pool(name=pn("dram"), bufs=1, space="DRAM") as dram,
    ):
        eps_sb = singles.tile([P, 1], F32)
        nc.vector.memset(eps_sb, eps)
        ones_sb = singles.tile([P, HD], F32)
        nc.vector.memset(ones_sb, 1.0)
        mask_sb = None
        if "mask" in flags:
            mask_sb = singles.tile([P, KT], F32)
            nc.gpsimd.dma_start(mask_sb,
                                opt["mask"].rearrange("(a p) -> p a", p=P))

        def col_strip(name, n):
            if name not in flags:
                return None
            t = singles.tile([P, n // P], F32, tag=f"strip_{name}")
            nc.gpsimd.dma_start(t, opt[name].rearrange("(a p) -> p a", p=P))
            return t
        bq_sb = col_strip("bq", H)
        bk_sb = col_strip("bk", H)
        b1_sb = col_strip("b1", FF)
        b1x_sb = None
        if b1_sb is not None:
            b1x_sb = singles.tile([P, FF // P], F32)
            nc.vector.tensor_scalar_mul(b1x_sb, b1_sb, SCALE * SCALE)

        def rep_row(name, n):
            if name not in flags:
                return None
            t = singles.tile([P, n], F32, tag=f"rep_{name}")
            src = opt[name][:]
            bcast = bass.AP(tensor=src.tensor, offset=src.offset,
                            ap=[[0, P]] + list(src.ap))
            nc.gpsimd.dma_start(t, bcast)
            return t
        bv_sb = rep_row("bv", H)
        b2_sb = rep_row("b2", H)
        ln1g_sb = rep_row("ln1_g", H)
        ln1b_sb = rep_row("ln1_b", H)
        ln2g_sb = rep_row("ln2_g", H)
        ln2b_sb = rep_row("ln2_b", H)

        # persistent tensors (live through C)
        x1T = singles.tile([P, HT, SQ], BF16)     # 16*x1, transposed
        ctxT = singles.tile([P, HT, SQ], FP8)     # normalized ctx, transposed
        wo_sb = singles.tile([P, HT, H], FP8)
        w2_sb = singles.tile([P, FT, H], BF16)
        x1_dram = dram.tile([SQ, H], BF16)        # 16*x1 for the LN2 residual

        nc.gpsimd.dma_start(wo_sb, wo.rearrange("(a p) h -> p a h", p=P))
        nc.gpsimd.dma_start(w2_sb, w2.rearrange("(a p) h -> p a h", p=P))

        SG = 512
        NSG = H // SG
        U32 = mybir.dt.uint32

        def dve_rsqrt(pool, var_sl, n, out_scale):
            """DVE-only out_scale*16/sqrt(var_sl + eps) over [P, n]; no ACT
            table switch.  The x16 data scaling puts var/256 tightly near
            1.0, so a linear seed + 3 Newton iterations reaches ~1e-9 rel
            error; out_scale folds into the last iteration's constants."""
            va = pool.tile([P, n], F32, tag="rsq_v", name="va")
            r = pool.tile([P, n], F32, tag="rsq_r", name="rr")
            a = pool.tile([P, n], F32, tag="rsq_a", name="aa")
            nc.vector.tensor_scalar(va, var_sl, 1.0 / (SCALE * SCALE), eps,
                                    ALU.mult, ALU.add)
            nc.vector.tensor_scalar(r, va, -0.5, 1.5, ALU.mult, ALU.add)
            for it in range(3):
                nc.vector.tensor_tensor(a, r, r, ALU.mult)
                nc.vector.tensor_tensor(a, a, va, ALU.mult)
                c = out_scale if it == 2 else 1.0
                nc.vector.tensor_scalar(a, a, -0.5 * c, 1.5 * c,
                                        ALU.mult, ALU.add)
                nc.vector.tensor_tensor(r, r, a, ALU.mult)
            return r

        # ---------------- phases A+B: projections + attention -------------
        with (
            tc.tile_pool(name=pn("qkvdata"), bufs=1) as qkvd,
        ):
            qT = qkvd.tile([P, HT, SQ], FP8)
            kT = qkvd.tile([P, HT, S], FP8)
            v_sb = qkvd.tile([P, KT, NH, HD + 2], FP8)
            xc_full = qkvd.tile([P, HT, S], FP8)
            wv_sb = qkvd.tile([P, HT, H], FP8)
            nc.vector.memset(v_sb, 1.0)
            nc.gpsimd.dma_start(wv_sb, wv.rearrange("(a p) d -> p a d", p=P))

            # phase A: Q + K projections (fp8 DoubleRow); V is deferred into
            # the attention slot loop as PE filler.
            with (
                tc.tile_pool(name=pn("aw"), bufs=1) as awp,
                tc.tile_pool(name=pn("xq"), bufs=2) as xqp,
                tc.tile_pool(name=pn("a_ps"), bufs=4, space="PSUM") as qps,
            ):
                wq_sb = awp.tile([P, HT, H], FP8)
                wk_sb = awp.tile([P, HT, H], FP8)
                nc.sync.dma_start(wq_sb, wq.rearrange("(a p) d -> p a d", p=P))
                nc.scalar.dma_start(wk_sb,
                                    wk.rearrange("(a p) d -> p a d", p=P))
                xqTr = xqT.rearrange("(a p) t -> p a t", p=P)
                for tci in range(SQ // TC):
                    t_sl = slice(tci * TC, (tci + 1) * TC)
                    xq_c = xqp.tile([P, HT, TC], FP8, tag="xq")
                    nc.sync.dma_start(xq_c, xqTr[:, :, t_sl])
                    for dt in range(HT):
                        ps = qps.tile([P, TC], F32, tag="q")
                        for ht in range(0, HT, 2):
                            nc.tensor.matmul(
                                ps, wq_sb[:, ht:ht + 2, dt * P:(dt + 1) * P],
                                xq_c[:, ht:ht + 2, :],
                                start=(ht == 0), stop=(ht == HT - 2),
                                perf_mode=DR)
                        d_sl = qT[:, dt, t_sl]
                        if bq_sb is not None:
                            nc.vector.tensor_scalar(
                                d_sl, ps, bq_sb[:, dt:dt + 1], QK_SCALE,
                                ALU.add, ALU.mult)
                        else:
                            nc.vector.tensor_scalar_mul(d_sl, ps, QK_SCALE)

                xTr = xT.rearrange("(a p) t -> p a t", p=P)
                for tci in range(S // TC):
                    t_sl = slice(tci * TC, (tci + 1) * TC)
                    eng = nc.scalar if tci % 2 == 0 else nc.sync
                    eng.dma_start(xc_full[:, :, t_sl], xTr[:, :, t_sl])
                    for dt in range(HT):
                        ps = qps.tile([P, TC], F32, tag="q")
                        for ht in range(0, HT, 2):
                            nc.tensor.matmul(
                                ps, wk_sb[:, ht:ht + 2, dt * P:(dt + 1) * P],
                                xc_full[:, ht:ht + 2, t_sl],
                                start=(ht == 0), stop=(ht == HT - 2),
                                perf_mode=DR)
                        d_sl = kT[:, dt, t_sl]
                        if bk_sb is not None:
                            nc.vector.tensor_scalar(
                                d_sl, ps, bk_sb[:, dt:dt + 1], QK_SCALE,
                                ALU.add, ALU.mult)
                        else:
                            nc.vector.tensor_scalar_mul(d_sl, ps, QK_SCALE)

            # phase B: attention. Softmax exps on ACT are the critical path;
            # the PE slot loop interleaves scores, lagged ctx, and filler
            # work (V projection, then the previous chunk's Wo+LN1).
            with (
                tc.tile_pool(name=pn("probs"), bufs=3) as probsp,
                tc.tile_pool(name=pn("stage"), bufs=4) as stagep,
                tc.tile_pool(name=pn("cstp"), bufs=4) as cstp,
                tc.tile_pool(name=pn("rsp"), bufs=2) as rsp,
                tc.tile_pool(name=pn("ln1"), bufs=2) as ln1p,
                tc.tile_pool(name=pn("ybuf"), bufs=1) as ybufp,
                tc.tile_pool(name=pn("sc_ps"), bufs=2, space="PSUM") as scp,
                tc.tile_pool(name=pn("ctx_ps"), bufs=2, space="PSUM") as ctxp,
                tc.tile_pool(name=pn("wo_ps"), bufs=2, space="PSUM") as wops,
            ):
                fillers = []

                def v_filler(tt, dh):
                    def emit():
                        ps = wops.tile([P, 512], F32, tag="wo")
                        for ht in range(0, HT, 2):
                            nc.tensor.matmul(
                                ps, xc_full[:, ht:ht + 2, tt * P:(tt + 1) * P],
                                wv_sb[:, ht:ht + 2, dh * 512:(dh + 1) * 512],
                                start=(ht == 0), stop=(ht == HT - 2),
                                perf_mode=DR)
                        if bv_sb is not None:
                            nc.vector.tensor_tensor(
                                ps, ps, bv_sb[:, dh * 512:(dh + 1) * 512],
                                ALU.add)
                        nc.vector.tensor_scalar_mul(
                            v_sb[:, tt, dh * 8:(dh + 1) * 8, 0:HD],
                            ps.rearrange("p (nh hd) -> p nh hd", hd=HD),
                            1.0 / SCALE)
                    return emit

                # V fillers in k-tile order: ctx(hp, g) needs tiles
                # (tt=G*g..G*g+3, dh=hp//4); popping 4 per slot early in
                # chunk 0 stays ahead of the lagged ctx consumer.
                for tt in range(KT):
                    fillers.append(("v", v_filler(tt, 0)))
                for tt in range(KT):
                    fillers.append(("v", v_filler(tt, 1)))

                def wo_ln1_items(qc):
                    """Wo + residual per token tile of chunk qc, then one
                    batched LN1 (a single sqrt table visit per chunk)."""
                    ybuf = ybufp.tile([P, NTT, H], BF16, tag="y",
                                      name=f"ybuf{qc}")
                    mvb = ybufp.tile([P, NTT, 2], F32, tag="mvb",
                                     name=f"mvb{qc}")
                    xh_ts = {}

                    def wo_item(tt, hoc):
                        def emit():
                            tok0 = qc * QC + tt * P
                            if hoc == 0:
                                xh_t = ln1p.tile([P, H], BF16, tag="xh")
                                nc.sync.dma_start(xh_t, xh[tok0:tok0 + P, :])
                                xh_ts[tt] = xh_t
                            o_sl = slice(hoc * 512, (hoc + 1) * 512)
                            ps_a = wops.tile([P, 512], F32, tag="wo")
                            for st in range(0, HT, 2):
                                nc.tensor.matmul(
                                    ps_a,
                                    ctxT[:, st:st + 2, tok0:tok0 + P],
                                    wo_sb[:, st:st + 2, o_sl],
                                    start=(st == 0), stop=(st == HT - 2),
                                    perf_mode=DR)
                            nc.vector.tensor_tensor(
                                ybuf[:, tt, o_sl], ps_a,
                                xh_ts[tt][:, o_sl], ALU.add)
                            if hoc == 1:
                                st6 = ln1p.tile([P, NSG, 6], F32, tag="st6")
                                for sg in range(NSG):
                                    nc.vector.bn_stats(
                                        st6[:, sg, :],
                                        ybuf[:, tt, sg * SG:(sg + 1) * SG])
                                nc.vector.bn_aggr(mvb[:, tt, :], st6)
                        return emit

                    def ln1_finish():
                        r = dve_rsqrt(ln1p, mvb[:, :, 1], NTT, 1.0)
                        for tt in range(NTT):
                            tok0 = qc * QC + tt * P
                            x1b_t = ln1p.tile([P, H], BF16, tag="x1b")
                            nc.vector.tensor_scalar(
                                x1b_t, ybuf[:, tt, :], mvb[:, tt, 0:1],
                                r[:, tt:tt + 1],
                                ALU.subtract, ALU.mult)
                            if ln1g_sb is not None:
                                nc.vector.tensor_tensor(x1b_t, x1b_t,
                                                        ln1g_sb, ALU.mult)
                            if ln1b_sb is not None:
                                nc.vector.tensor_tensor(x1b_t, x1b_t,
                                                        ln1b_sb, ALU.add)
                            nc.sync.dma_start(x1_dram[tok0:tok0 + P, :],
                                              x1b_t)
                            nc.sync.dma_start_transpose(
                                x1T[:, :, tok0:tok0 + P], x1b_t)

                    items = [wo_item(tt, hoc)
                             for tt in range(NTT) for hoc in range(2)]
                    items.append(ln1_finish)
                    return items

                for qc in range(NQC):
                    q_sl = slice(qc * QC, (qc + 1) * QC)
                    pending = []
                    norm_q = []
                    pcx = {}
                    staged = {}
                    rs_cur = [None]

                    def emit_ctx(chp, cg, pr, qc=qc, q_sl=q_sl, pcx=pcx,
                                 staged=staged, rs_cur=rs_cur):
                        if cg == 0:
                            pce = ctxp.tile([P, QC], F32, tag="ctx")
                            pco = ctxp.tile([P, QC], F32, tag="ctx")
                            pcx[chp] = (pce, pco)
                        pce, pco = pcx[chp]
                        for par, pc in ((0, pce), (1, pco)):
                            h = 2 * chp + par
                            for jj in (0, 2):
                                nc.tensor.matmul(
                                    pc[0:HD + 1, :],
                                    v_sb[:, G * cg + jj:G * cg + jj + 2,
                                         h, 0:HD + 1],
                                    pr[:, jj:jj + 2, par, :],
                                    start=(cg == 0 and jj == 0),
                                    stop=(cg == NG - 1 and jj == 2),
                                    perf_mode=DR)
                        if cg == NG - 1:
                            if chp % 2 == 0:
                                rs_cur[0] = rsp.tile([P, QC], F32, tag="rs",
                                                     name="rs_t")
                            rs_t = rs_cur[0]
                            cs_pair = []
                            for par, pc in ((0, pce), (1, pco)):
                                cs = stagep.tile([P, QC], BF16, tag="cs")
                                nc.vector.tensor_copy(cs[0:HD + 1, :],
                                                      pc[0:HD + 1, :])
                                cs_pair.append(cs)
                                rrow = 32 * (2 * (chp % 2) + par)
                                nc.gpsimd.dma_start(
                                    rs_t[rrow:rrow + 1, :],
                                    cs[HD:HD + 1, :])
                            staged[chp] = cs_pair
                            del pcx[chp]
                            if chp % 2 == 1:
                                nc.vector.reciprocal(rs_t, rs_t)

                                def norm_pair(php, rs_t=rs_t, q_sl=q_sl,
                                              staged=staged):
                                    for par in range(2):
                                        r = 32 * (2 * (php % 2) + par)
                                        rb = wops.tile([P, QC], F32, tag="wo")
                                        nc.tensor.matmul(
                                            rb[0:HD, :],
                                            ones_sb[r:r + 1, 0:HD],
                                            rs_t[r:r + 1, :],
                                            start=True, stop=True,
                                            tile_position=(r, 0))
                                        cs = staged[php][par]
                                        cst = cstp.tile([P, QC], FP8,
                                                        tag="cst")
                                        nc.vector.tensor_tensor(
                                            cst[0:HD, :], cs[0:HD, :],
                                            rb[0:HD, :], ALU.mult)
                                        phs = par * HD
                                        nc.sync.dma_start(
                                            ctxT[phs:phs + HD, php, q_sl],
                                            cst[0:HD, :])
                                    del staged[php]
                                for php in (chp - 1, chp):
                                    norm_pair(php)

                    slot_idx = [0]
                    for hp in range(NP + 1):
                        for g in range(NG):
                            slot_idx[0] = hp * NG + g
                            if hp < NP:
                                pr = probsp.tile([P, G, 2, QC], FP8,
                                                 tag="probs")
                                for lkt in range(G):
                                    kt = G * g + lkt
                                    ps_s = scp.tile([P, 2, QC], F32, tag="sc")
                                    for par in range(2):
                                        hs = par * HD
                                        nc.tensor.matmul(
                                            ps_s[:, par, :],
                                            kT[hs:hs + HD, hp,
                                               kt * P:(kt + 1) * P],
                                            qT[hs:hs + HD, hp, q_sl],
                                            start=True, stop=True)
                                    if mask_sb is not None:
                                        mvw = mask_sb[:, kt:kt + 1, None]
                                        nc.vector.tensor_tensor(
                                            ps_s, ps_s,
                                            mvw.to_broadcast((P, 2, QC)),
                                            ALU.add)
                                    nc.scalar.activation(
                                        pr[:, lkt, :, :], ps_s, AF.Exp)
                                pending.append((hp, g, pr))
                            # fillers before ctx so ctx's V deps exist;
                            # drain the backlog at a slots-left pace
                            npop = 4 if (qc == 0 and hp < 2) else 1
                            for _ in range(npop):
                                if not fillers:
                                    break
                                fillers.pop(0)[1]()
                            if len(pending) >= 2 or (hp == NP and pending):
                                chp, cg, cpr = pending.pop(0)
                                emit_ctx(chp, cg, cpr)
                            if hp == NP:
                                break

                    # queue this chunk's Wo+LN1 (and for chunk 0, its FFN
                    # W1+gelu) as fillers for the next chunk; for the last
                    # chunk, emit directly.
                    items = wo_ln1_items(qc)
                    if qc < NQC - 1:
                        fillers.extend(("wo", it) for it in items)
                    else:
                        while fillers:
                            fillers.pop(0)[1]()
                        for it in items:
                            it()

        # ---------------- phase C: FFN (bf16) + LN2 -----------------------
        with (
            tc.tile_pool(name=pn("w1q"), bufs=2) as w1qp,
            tc.tile_pool(name=pn("gtp"), bufs=1) as gtp,
            tc.tile_pool(name=pn("y2p"), bufs=1) as y2p,
            tc.tile_pool(name=pn("ln2"), bufs=2) as ln2p,
            tc.tile_pool(name=pn("h_ps"), bufs=3, space="PSUM") as hps,
            tc.tile_pool(name=pn("o_ps"), bufs=3, space="PSUM") as ops,
        ):
            gt = gtp.tile([P, FT, QC], BF16)
            y2buf = y2p.tile([P, NTT, H], F32)
            st6buf = y2p.tile([P, NTT, NSG, 6], F32)
            FQ = FF // 4
            for ch in range(NQC):
                t_sl = slice(ch * QC, (ch + 1) * QC)
                # W1 + gelu, streaming w1 in quarters (bf16)
                for fq in range(4):
                    w1q = w1qp.tile([P, HT, FQ], BF16, tag="w1q")
                    nc.gpsimd.dma_start(
                        w1q, w1[:, fq * FQ:(fq + 1) * FQ]
                        .rearrange("(a p) f -> p a f", p=P))
                    for lf in range(FQ // P):
                        f = fq * (FQ // P) + lf
                        ps = hps.tile([P, QC], F32, tag="h")
                        for ht in range(HT):
                            nc.tensor.matmul(
                                ps, w1q[:, ht, lf * P:(lf + 1) * P],
                                x1T[:, ht, t_sl],
                                start=(ht == 0), stop=(ht == HT - 1))
                        bias = (b1_sb[:, f:f + 1]
                                if b1_sb is not None else 0.0)
                        nc.scalar.activation(gt[:, f, :], ps,
                                             AF.Gelu_apprx_tanh, bias=bias,
                                             scale=1.0 / (SCALE * SCALE))
                # W2 (w2 resident bf16, full-FF accumulation in PSUM) + y2
                mvs = []
                for tt in range(NTT):
                    tok0 = ch * QC + tt * P
                    x1l2_t = ln2p.tile([P, H], BF16, tag="x1l2")
                    nc.sync.dma_start(x1l2_t, x1_dram[tok0:tok0 + P, :])
                    for hoc in range(2):
                        o_sl = slice(hoc * 512, (hoc + 1) * 512)
                        ps2 = ops.tile([P, 512], F32, tag="o")
                        for m in range(FT):
                            nc.tensor.matmul(
                                ps2, gt[:, m, tt * P:(tt + 1) * P],
                                w2_sb[:, m, o_sl],
                                start=(m == 0), stop=(m == FT - 1))
                        nc.vector.tensor_tensor(y2buf[:, tt, o_sl], ps2,
                                                x1l2_t[:, o_sl], ALU.add)
                    if b2_sb is not None:
                        nc.vector.tensor_tensor(y2buf[:, tt, :],
                                                y2buf[:, tt, :], b2_sb,
                                                ALU.add)
                    for sg in range(NSG):
                        nc.vector.bn_stats(st6buf[:, tt, sg, :],
                                           y2buf[:, tt, sg * SG:(sg + 1) * SG])
                    mv = ln2p.tile([P, 2], F32, tag="mv2")
                    nc.vector.bn_aggr(mv, st6buf[:, tt, :, :])
                    mvs.append(mv)
                # batched LN2 (one sqrt table visit per chunk)
                for mv in mvs:
                    nc.scalar.activation(mv[:, 1:2], mv[:, 1:2], AF.Sqrt,
                                         bias=eps_sb)
                for mv in mvs:
                    nc.vector.reciprocal(mv[:, 1:2], mv[:, 1:2])
                for tt in range(NTT):
                    tok0 = ch * QC + tt * P
                    mv = mvs[tt]
                    o_t = ln2p.tile([P, H], F32, tag="o")
                    nc.vector.tensor_scalar(o_t, y2buf[:, tt, :], mv[:, 0:1],
                                            mv[:, 1:2], ALU.subtract,
                                            ALU.mult)
                    if ln2g_sb is not None:
                        nc.vector.tensor_tensor(o_t, o_t, ln2g_sb, ALU.mult)
                    if ln2b_sb is not None:
                        nc.vector.tensor_tensor(o_t, o_t, ln2b_sb, ALU.add)
                    nc.sync.dma_start(out_ext[tok0:tok0 + P, :], o_t)


# ---------------------------------------------------------------------------
# host side
# ---------------------------------------------------------------------------

def _nonzero(a):
    return bool(np.any(np.asarray(a) != 0))


def compute_flags(inputs):
    flags = set()
    if _nonzero(inputs["attention_mask"]):
        flags.add("mask")
    for name in ["bq", "bk", "bv", "b1", "b2", "ln1_b", "ln2_b"]:
        if _nonzero(inputs[name]):
            flags.add(name)
    for name in ["ln1_g", "ln2_g"]:
        if bool(np.any(np.asarray(inputs[name]) != 1)):
            flags.add(name)
    return flags


def make_in_maps(S, H, FF, inputs, flags):
    """Shard full inputs into 8 per-core input maps."""
    import ml_dtypes
    fp8 = ml_dtypes.float8_e4m3
    bf16 = ml_dtypes.bfloat16
    SQ = S // 2
    x = np.asarray(inputs["x"], np.float32)       # [4, S, H]
    bo = np.asarray(inputs["bo"], np.float32)

    def w8(name):
        w = np.asarray(inputs[name], np.float32) * SCALE
        return np.ascontiguousarray(w).astype(fp8)

    def w16(name):
        w = np.asarray(inputs[name], np.float32) * SCALE
        return np.ascontiguousarray(w).astype(bf16)

    shared = {
        "wq": w8("Wq"), "wk": w8("Wk"), "wv": w8("Wv"), "wo": w8("Wo"),
        "w1": w16("W1"), "w2": w16("W2"),
    }
    scale16 = {"bq", "bk", "bv", "b2", "ln1_b"}
    for name in ["bq", "bk", "bv", "b1", "b2",
                 "ln1_g", "ln1_b", "ln2_g", "ln2_b"]:
        if name in flags:
            v = np.asarray(inputs[name], np.float32)
            if name in scale16:
                v = v * SCALE
            shared[name] = np.ascontiguousarray(v)
    xT_by_batch = [np.ascontiguousarray(x[b].T).astype(fp8) for b in range(4)]
    maps = []
    for c in range(8):
        b, j = divmod(c, 2)
        xTb = xT_by_batch[b]
        m = dict(shared)
        m["xT"] = xTb
        m["xqT"] = np.ascontiguousarray(xTb[:, j * SQ:(j + 1) * SQ])
        m["xh"] = np.ascontiguousarray(
            SCALE * (x[b, j * SQ:(j + 1) * SQ] + bo[None, :]),
            ).astype(bf16)
        if "mask" in flags:
            m["mask"] = np.ascontiguousarray(
                np.asarray(inputs["attention_mask"], np.float32)[b, 0, 0, :])
        maps.append(m)
    return maps


LAST_EXEC_NS = None
LAST_RESULTS = None


def _install_ntff_hook():
    """Register the NTFF profiling hook (missing antenv.axon_hooks shim)."""
    if "antenv.axon_hooks" in sys.modules:
        return
    try:
        import antenv  # noqa: F401
        mod = types.ModuleType("antenv.axon_hooks")
        hook = [None]
        mod.set_axon_ntff_profile_hook = lambda h: hook.__setitem__(0, h)
        mod.get_axon_ntff_profile_hook = lambda: hook[0]
        sys.modules["antenv.axon_hooks"] = mod
        from trn_agent_boot.trn_boot import _ntff_profile_via_ctypes
        mod.set_axon_ntff_profile_hook(
            _ntff_profile_via_ctypes("/opt/axon/libaxon_pjrt.so"))
    except Exception:
        sys.modules.pop("antenv.axon_hooks", None)


def run_block(S, H, FF, inputs, trace=False):
    """Build, compile, run on 8 cores; returns [B, S, H] output."""
    global LAST_EXEC_NS, LAST_RESULTS
    flags = compute_flags(inputs)
    nc = bacc.Bacc("TRN2", target_bir_lowering=False, debug=True)
    build_block(nc, S=S, H=H, NH=16, FF=FF, flags=flags)
    nc.compile()
    in_maps = make_in_maps(S, H, FF, inputs, flags)
    if trace:
        _install_ntff_hook()
    res = run_bass_kernel_spmd(
        nc, in_maps, core_ids=list(range(8)), trace=trace,
        trace_cores=[0] if trace else None)
    LAST_EXEC_NS = res.exec_time_ns
    LAST_RESULTS = res
    SQ = S // 2
    B = 4
    out = np.empty((B, S, H), np.float32)
    for c in range(8):
        b, j = divmod(c, 2)
        out[b, j * SQ:(j + 1) * SQ] = res.results[c]["out"]
    return out


def kernel(x, attention_mask, Wq, bq, Wk, bk, Wv, bv, Wo, bo,
           ln1_g, ln1_b, W1, b1, W2, b2, ln2_g, ln2_b):
    inputs = dict(x=x, attention_mask=attention_mask, Wq=Wq, bq=bq, Wk=Wk,
                  bk=bk, Wv=Wv, bv=bv, Wo=Wo, bo=bo, ln1_g=ln1_g,
                  ln1_b=ln1_b, W1=W1, b1=b1, W2=W2, b2=b2, ln2_g=ln2_g,
                  ln2_b=ln2_b)
    trace = bool(int(os.environ.get("BLOCK_TRACE", "0")))
    return run_block(2048, 1024, 4096, inputs, trace=trace)
